# revision 2
# baseline (speedup 1.0000x reference)
"""Bispectrum on S1xS1 — Trainium2 Bass kernel.

Full-input contract: kernel(x) with x (2, 64, 64) float32 returns
B (2, 4096, 4096) complex64 where, with X = fft2(x),
  B[b, (i,j), (p,q)] = X[b,i,j] * X[b,p,q] * conj(X[b,(i+p)%64,(j+q)%64]).

Sharding: 8 cores = 2 batches x 4 row-quarters. Each core computes a
(1024, 4096) complex row-block of its batch's B:
  - tiny 64-pt DFTs on PE (host passes DFT matrices as constants; a
    row-rotated copy of the spectrum folds the core's row-offset into
    per-core constant data so the SPMD program has no core-dependent APs)
  - rank-2 matmuls on PE build the complex outer product U = a x b
  - a sliding-window DMA over a doubled spectrum builds the stack of
    rolled-spectrum circulant blocks C in SBUF
  - DVE/GpSimd combine Re/Im = U * conj(C) into an interleaved f32 tile
    that DMAs out as complex64 memory layout.
"""

import os
import sys

for _p in ("/opt/trn_rl_repo", "/opt/pypackages"):
    if _p not in sys.path:
        sys.path.insert(0, _p)

import numpy as np

M = 64
MN = M * M
NCORES = 8
QUARTERS = 4
ROWS_PER_CORE = 2 * MN // NCORES  # 1024

_CACHE = {}


def _build_nc():
    import concourse.bass as bass
    import concourse.bacc as bacc
    import concourse.mybir as mybir
    from concourse.tile import TileContext

    f32 = mybir.dt.float32
    nc = bacc.Bacc("TRN2")

    x = nc.declare_dram_parameter("x", [M, M], f32, isOutput=False)
    fr = nc.declare_dram_parameter("fr", [M, M], f32, isOutput=False)
    fi = nc.declare_dram_parameter("fi", [M, M], f32, isOutput=False)
    fin = nc.declare_dram_parameter("fin", [M, M], f32, isOutput=False)
    frr = nc.declare_dram_parameter("frr", [M, M], f32, isOutput=False)
    fir = nc.declare_dram_parameter("fir", [M, M], f32, isOutput=False)
    finr = nc.declare_dram_parameter("finr", [M, M], f32, isOutput=False)
    out = nc.declare_dram_parameter("out", [ROWS_PER_CORE, 2 * MN], f32, isOutput=True)

    br_d = nc.dram_tensor("br_d", [MN], f32)
    bi_d = nc.dram_tensor("bi_d", [MN], f32)
    ar_d = nc.dram_tensor("ar_d", [1024], f32)
    ai_d = nc.dram_tensor("ai_d", [1024], f32)
    ain_d = nc.dram_tensor("ain_d", [1024], f32)
    xddr = nc.dram_tensor("xddr", [73, 128], f32)
    xddi = nc.dram_tensor("xddi", [73, 128], f32)

    with TileContext(nc) as tc:
        with (
            tc.tile_pool(name="const", bufs=1) as cp,
            tc.tile_pool(name="big", bufs=1) as bp,
            tc.tile_pool(name="tmp", bufs=3) as tp,
            tc.tile_pool(name="chunkp", bufs=3) as kp,
            tc.tile_pool(name="psum", bufs=2, space="PSUM") as pp,
        ):
            ACT = mybir.ActivationFunctionType

            def sb64(name_src):
                t = cp.tile([M, M], f32, tag=name_src.name)
                nc.sync.dma_start(out=t, in_=name_src[:, :])
                return t

            x_sb = sb64(x)
            fr_sb = sb64(fr)
            fi_sb = sb64(fi)
            fin_sb = sb64(fin)
            frr_sb = sb64(frr)
            fir_sb = sb64(fir)
            finr_sb = sb64(finr)

            # x^T via 32x32 stream-transpose blocks
            xt_sb = cp.tile([M, M], f32, tag="xt")
            for bi_ in range(2):
                for bj in range(2):
                    nc.vector.transpose(
                        xt_sb[bi_ * 32 : (bi_ + 1) * 32, bj * 32 : (bj + 1) * 32],
                        x_sb[bj * 32 : (bj + 1) * 32, bi_ * 32 : (bi_ + 1) * 32],
                    )

            # stage 1: W = x @ F
            wr_ps = pp.tile([M, M], f32, tag="fft")
            nc.tensor.matmul(wr_ps[:, :], lhsT=xt_sb, rhs=fr_sb, start=True, stop=True)
            wr_sb = cp.tile([M, M], f32, tag="wr")
            nc.scalar.copy(wr_sb, wr_ps)
            wi_ps = pp.tile([M, M], f32, tag="fft")
            nc.tensor.matmul(wi_ps[:, :], lhsT=xt_sb, rhs=fi_sb, start=True, stop=True)
            wi_sb = cp.tile([M, M], f32, tag="wi")
            nc.scalar.copy(wi_sb, wi_ps)

            # stage 2 (unrotated): X = F @ W, b-side spectrum
            def mm2(lhs1, rhs1, lhs2, rhs2_, tagn):
                ps = pp.tile([M, M], f32, tag="fft")
                nc.tensor.matmul(ps[:, :], lhsT=lhs1, rhs=rhs1, start=True, stop=False)
                nc.tensor.matmul(ps[:, :], lhsT=lhs2, rhs=rhs2_, start=False, stop=True)
                sb = cp.tile([M, M], f32, tag=tagn)
                nc.scalar.copy(sb, ps)
                return sb

            xr_sb = mm2(fr_sb, wr_sb, fin_sb, wi_sb, "xr")
            xi_sb = mm2(fr_sb, wi_sb, fi_sb, wr_sb, "xi")
            # stage 2 (rotated by quarter*16 rows, folded into host consts)
            xrr_sb = mm2(frr_sb, wr_sb, finr_sb, wi_sb, "xrr")
            xri_sb = mm2(frr_sb, wi_sb, fir_sb, wr_sb, "xri")

            # b-side flats: rhs2 rows = [Xr_flat, Xi_flat]
            nc.sync.dma_start(out=br_d.rearrange("(p f) -> p f", p=M), in_=xr_sb)
            nc.sync.dma_start(out=bi_d.rearrange("(p f) -> p f", p=M), in_=xi_sb)
            rhs2 = bp.tile([2, MN], f32, tag="rhs2")
            nc.sync.dma_start(out=rhs2[0:1, :], in_=br_d.rearrange("(p f) -> p f", p=1))
            nc.sync.dma_start(out=rhs2[1:2, :], in_=bi_d.rearrange("(p f) -> p f", p=1))

            # a-side: rows 0:16 of rotated spectrum = this core's 16 global rows
            nc.sync.dma_start(
                out=ar_d.rearrange("(p f) -> p f", p=16), in_=xrr_sb[0:16, :]
            )
            nc.sync.dma_start(
                out=ai_d.rearrange("(p f) -> p f", p=16), in_=xri_sb[0:16, :]
            )
            ain_sb = cp.tile([16, M], f32, tag="ain")
            nc.vector.tensor_scalar_mul(ain_sb, xri_sb[0:16, :], -1.0)
            nc.sync.dma_start(
                out=ain_d.rearrange("(p f) -> p f", p=16), in_=ain_sb
            )
            xa = bp.tile([2, 1024], f32, tag="xa")  # [ar; -ai] -> Ur
            nc.sync.dma_start(out=xa[0:1, :], in_=ar_d.rearrange("(p f) -> p f", p=1))
            nc.sync.dma_start(out=xa[1:2, :], in_=ain_d.rearrange("(p f) -> p f", p=1))
            xb = bp.tile([2, 1024], f32, tag="xb")  # [ai; ar] -> Ui
            nc.sync.dma_start(out=xb[0:1, :], in_=ai_d.rearrange("(p f) -> p f", p=1))
            nc.sync.dma_start(out=xb[1:2, :], in_=ar_d.rearrange("(p f) -> p f", p=1))

            # doubled rotated spectrum in DRAM: xdd[r, c] = Xrot[r%64, c%64]
            for (xdd, src_sb) in ((xddr, xrr_sb), (xddi, xri_sb)):
                nc.sync.dma_start(out=xdd[0:64, 0:64], in_=src_sb)
                nc.sync.dma_start(out=xdd[0:64, 64:128], in_=src_sb)
                nc.sync.dma_start(out=xdd[64:73, 0:64], in_=src_sb[0:9, :])
                nc.sync.dma_start(out=xdd[64:73, 64:128], in_=src_sb[0:9, :])

            # circulant stacks: call[(s,j), (v,q)] = xdd[v+s, j+q]
            call_r = bp.tile([128, 72 * 64], f32, tag="call_r")
            call_i = bp.tile([128, 72 * 64], f32, tag="call_i")
            for (callt, xdd) in ((call_r, xddr), (call_i, xddi)):
                for s in range(2):
                    dest = callt[s * 64 : (s + 1) * 64, :].rearrange(
                        "j (v q) -> j v q", v=72
                    )
                    srcap = bass.AP(
                        tensor=xdd, offset=s * 128, ap=[[1, 64], [128, 72], [1, 64]]
                    )
                    nc.sync.dma_start(out=dest, in_=srcap)

            # main loop: 8 row-blocks x 8 column-chunks
            for gl in range(8):
                for pc in range(8):
                    v0 = (2 * gl + 8 * pc) % 64
                    ur = pp.tile([128, 512], f32, tag="ur")
                    ui = pp.tile([128, 512], f32, tag="ui")
                    nc.tensor.matmul(
                        ur[:, :],
                        lhsT=xa[:, gl * 128 : (gl + 1) * 128],
                        rhs=rhs2[:, pc * 512 : (pc + 1) * 512],
                        start=True,
                        stop=True,
                    )
                    nc.tensor.matmul(
                        ui[:, :],
                        lhsT=xb[:, gl * 128 : (gl + 1) * 128],
                        rhs=rhs2[:, pc * 512 : (pc + 1) * 512],
                        start=True,
                        stop=True,
                    )
                    cr = call_r[:, v0 * 64 : v0 * 64 + 512]
                    ci = call_i[:, v0 * 64 : v0 * 64 + 512]
                    t1 = tp.tile([128, 512], f32, tag="t1")
                    t2 = tp.tile([128, 512], f32, tag="t2")
                    t3 = tp.tile([128, 512], f32, tag="t3")
                    t4 = tp.tile([128, 512], f32, tag="t4")
                    nc.vector.tensor_mul(t1, ur, cr)
                    nc.vector.tensor_mul(t2, ui, ci)
                    nc.vector.tensor_mul(t3, ui, cr)
                    nc.vector.tensor_mul(t4, ur, ci)
                    chunk = kp.tile([128, 512, 2], f32, tag="chunk")
                    nc.gpsimd.tensor_add(chunk[:, :, 0], t1, t2)
                    nc.vector.tensor_sub(chunk[:, :, 1], t3, t4)
                    nc.sync.dma_start(
                        out=out[
                            gl * 128 : (gl + 1) * 128, pc * 1024 : (pc + 1) * 1024
                        ].rearrange("r (c two) -> r c two", two=2),
                        in_=chunk[:, :, :],
                    )
    nc.compile()
    return nc


def _dft_consts():
    k = np.arange(M)
    ang = -2.0 * np.pi * np.outer(k, k) / M
    Fr = np.cos(ang).astype(np.float32)
    Fi = np.sin(ang).astype(np.float32)
    return Fr, Fi


def _in_maps(x):
    Fr, Fi = _dft_consts()
    FiN = np.ascontiguousarray(-Fi)
    maps = []
    for core in range(NCORES):
        b = core // QUARTERS
        q = core % QUARTERS
        rFr = np.roll(Fr, -q * 16, axis=0)
        rFi = np.roll(Fi, -q * 16, axis=0)
        maps.append(
            {
                "x": np.ascontiguousarray(x[b]),
                "fr": Fr,
                "fi": Fi,
                "fin": FiN,
                "frr": np.ascontiguousarray(rFr.T),
                "fir": np.ascontiguousarray(rFi.T),
                "finr": np.ascontiguousarray(-rFi.T),
            }
        )
    return maps


def _assemble(results):
    out = np.empty((2, MN, MN), dtype=np.complex64)
    for core in range(NCORES):
        b = core // QUARTERS
        q = core % QUARTERS
        blk = np.asarray(results[core]["out"], dtype=np.float32)
        out[b, q * ROWS_PER_CORE : (q + 1) * ROWS_PER_CORE, :] = blk.view(
            np.complex64
        ).reshape(ROWS_PER_CORE, MN)
    return out


def kernel(x):
    from concourse.bass_utils import run_bass_kernel_spmd

    x = np.asarray(x, dtype=np.float32)
    if "nc" not in _CACHE:
        _CACHE["nc"] = _build_nc()
    nc = _CACHE["nc"]
    trace = os.environ.get("BISPEC_TRACE", "0") == "1"
    res = run_bass_kernel_spmd(
        nc, _in_maps(x), core_ids=list(range(NCORES)), trace=trace
    )
    _CACHE["last_exec_time_ns"] = res.exec_time_ns
    return _assemble(res.results)
